# revision 1
# baseline (speedup 1.0000x reference)
"""GCN (2-layer, symmetric-norm message passing) on 8 Trainium2 NeuronCores.

Contract: kernel(**inputs) takes the FULL inputs (x [50000,4,300] f32,
edge_index [2,250000] i32, W1/b1/W2/b2) and returns the FULL output
[50000,300] f32.

Strategy (per sharding hint): shard destination nodes across the 8 cores
(6250 each), replicate the small weights, partition edges by destination so
scatter-adds are core-local, and AllGather the pre-scaled source features
between layers.  The scatter-add itself is computed on the PE array as a
sequence of 0/1-indicator matmuls over 128-edge chunks (edges sorted by
destination on the host), with the per-row gather done by indirect DMA.
"""

import math

import numpy as np

import concourse.bacc as bacc
import concourse.bass as bass
import concourse.tile as tile
from concourse import bass_utils, mybir
from concourse.bass import IndirectOffsetOnAxis
from concourse.masks import make_identity

F32 = mybir.dt.float32
BF16 = mybir.dt.bfloat16
I32 = mybir.dt.int32
P = 128

N_CORES = 8


def _cdiv(a, b):
    return (a + b - 1) // b


# ---------------------------------------------------------------- host prep


def prep_inputs(x, edge_index, W1, b1, W2, b2, n_cores=N_CORES):
    """Shard + preprocess the full inputs into per-core in_maps.

    Returns (in_maps, meta) where meta carries the dims needed to build the
    device program.
    """
    N, T, C = x.shape
    assert N % n_cores == 0
    NPC = N // n_cores
    NBLK = _cdiv(NPC, P)

    row = np.asarray(edge_index[0], dtype=np.int64)
    col = np.asarray(edge_index[1], dtype=np.int64)

    # symmetric sqrt-degree norm; degree on source (row), +1 for self loops
    deg = (np.bincount(row, minlength=N) + 1).astype(np.float32)
    dis = (deg.astype(np.float32) ** -0.5).astype(np.float32)

    core_of = col // NPC

    # first pass: per-core per-block edge counts -> global CPB
    per_core = []
    max_blk = 0
    for c in range(n_cores):
        m = core_of == c
        r = row[m]
        d = col[m] - c * NPC
        order = np.argsort(d, kind="stable")
        r = r[order]
        d = d[order]
        cnt = np.bincount(d // P, minlength=NBLK)
        max_blk = max(max_blk, int(cnt.max()) if len(cnt) else 0)
        per_core.append((r, d, cnt))
    CPB = max(1, _cdiv(max_blk, P))

    # replicated tensors
    CC = [(c0, min(P, C - c0)) for c0 in range(0, C, P)]
    KC = len(CC)
    import ml_dtypes

    w1c = np.zeros((KC, P, C), ml_dtypes.bfloat16)
    w2c = np.zeros((KC, P, C), ml_dtypes.bfloat16)
    for k, (c0, cs) in enumerate(CC):
        w1c[k, :cs, :] = (W1.T[c0 : c0 + cs, :] / np.float32(T)).astype(np.float32)
        w2c[k, :cs, :] = W2.T[c0 : c0 + cs, :].astype(np.float32)
    b1t = np.broadcast_to(np.asarray(b1, np.float32), (P, C)).copy()
    b2t = np.broadcast_to(np.asarray(b2, np.float32), (P, C)).copy()
    iota = np.broadcast_to(np.arange(P, dtype=np.float32), (P, P)).copy()

    in_maps = []
    for c in range(n_cores):
        r, d, cnt = per_core[c]
        starts = np.concatenate([[0], np.cumsum(cnt)])
        idxt = np.zeros((P, NBLK * CPB), np.int32)
        dlt = np.full((P, NBLK * CPB), -1.0, np.float32)
        for blk in range(NBLK):
            s, e = int(starts[blk]), int(starts[blk + 1])
            n = e - s
            pad = CPB * P
            rb = np.zeros(pad, np.int64)
            rb[:n] = r[s:e]
            db = np.full(pad, -1.0, np.float32)
            db[:n] = (d[s:e] - blk * P).astype(np.float32)
            idxt[:, blk * CPB : (blk + 1) * CPB] = (
                rb.reshape(CPB, P).T.astype(np.int32)
            )
            dlt[:, blk * CPB : (blk + 1) * CPB] = db.reshape(CPB, P).T

        dis_c = dis[c * NPC : (c + 1) * NPC]
        dist = np.zeros((P, NBLK), np.float32)
        dist.reshape(-1)[: 0] = 0  # noop, keep shape
        flat = np.zeros(NBLK * P, np.float32)
        flat[:NPC] = dis_c
        dist[:, :] = flat.reshape(NBLK, P).T

        in_maps.append(
            {
                "xs": np.ascontiguousarray(x[c * NPC : (c + 1) * NPC]).astype(
                    np.float32
                ),
                "w1c": w1c,
                "w2c": w2c,
                "b1t": b1t,
                "b2t": b2t,
                "iot": iota,
                "dist": dist,
                "idxt": idxt,
                "dlt": dlt,
            }
        )

    meta = dict(N=N, T=T, C=C, NPC=NPC, NBLK=NBLK, CPB=CPB, CC=CC, n_cores=n_cores)
    return in_maps, meta


# ------------------------------------------------------------- device build


def build_nc(meta):
    N = meta["N"]
    T = meta["T"]
    C = meta["C"]
    NPC = meta["NPC"]
    NBLK = meta["NBLK"]
    CPB = meta["CPB"]
    CC = meta["CC"]
    KC = len(CC)
    n_cores = meta["n_cores"]
    rg = [list(range(n_cores))]

    nc = bacc.Bacc(
        "TRN2", target_bir_lowering=False, debug=False, num_devices=n_cores
    )

    xs = nc.dram_tensor("xs", [NPC, T, C], F32, kind="ExternalInput")
    w1c = nc.dram_tensor("w1c", [KC, P, C], BF16, kind="ExternalInput")
    w2c = nc.dram_tensor("w2c", [KC, P, C], BF16, kind="ExternalInput")
    b1t = nc.dram_tensor("b1t", [P, C], F32, kind="ExternalInput")
    b2t = nc.dram_tensor("b2t", [P, C], F32, kind="ExternalInput")
    iot = nc.dram_tensor("iot", [P, P], F32, kind="ExternalInput")
    dist = nc.dram_tensor("dist", [P, NBLK], F32, kind="ExternalInput")
    idxt = nc.dram_tensor("idxt", [P, NBLK * CPB], I32, kind="ExternalInput")
    dlt = nc.dram_tensor("dlt", [P, NBLK * CPB], F32, kind="ExternalInput")
    out_ext = nc.dram_tensor("out", [NPC, C], F32, kind="ExternalOutput")

    ACT = mybir.ActivationFunctionType

    with tile.TileContext(nc) as tc:
        with (
            tc.tile_pool(name="dramp", bufs=1, space="DRAM") as dramp,
            tc.tile_pool(name="singles", bufs=1) as singles,
            tc.tile_pool(name="work", bufs=3) as wp,
            tc.tile_pool(name="msgs", bufs=12) as mp,
            tc.tile_pool(name="psA", bufs=1, space="PSUM") as psA,
            tc.tile_pool(name="psT", bufs=2, space="PSUM") as psT,
            tc.tile_pool(name="psB", bufs=3, space="PSUM") as psB,
            tc.tile_pool(name="psC", bufs=2, space="PSUM") as psC,
        ):
            agin1 = dramp.tile([NPC, C], BF16, name="agin1")
            hp1f = dramp.tile([N, C], BF16, addr_space="Shared", name="hp1f")
            agin2 = dramp.tile([NPC, C], BF16, name="agin2")
            hp2f = dramp.tile([N, C], BF16, addr_space="Shared", name="hp2f")

            # constants / tables in SBUF
            ident = singles.tile([P, P], BF16, name="ident")
            make_identity(nc, ident[:])
            w1sb = singles.tile([P, KC, C], BF16, name="w1sb")
            w2sb = singles.tile([P, KC, C], BF16, name="w2sb")
            for k in range(KC):
                nc.sync.dma_start(out=w1sb[:, k, :], in_=w1c[k])
                nc.sync.dma_start(out=w2sb[:, k, :], in_=w2c[k])
            b1sb = singles.tile([P, C], F32, name="b1sb")
            nc.sync.dma_start(out=b1sb[:], in_=b1t[:])
            b2sb = singles.tile([P, C], F32, name="b2sb")
            nc.sync.dma_start(out=b2sb[:], in_=b2t[:])
            iosb = singles.tile([P, P], F32, name="iosb")
            nc.sync.dma_start(out=iosb[:], in_=iot[:])
            dissb = singles.tile([P, NBLK], F32, name="dissb")
            nc.sync.dma_start(out=dissb[:], in_=dist[:])
            idxsb = singles.tile([P, NBLK * CPB], I32, name="idxsb")
            nc.sync.dma_start(out=idxsb[:], in_=idxt[:])
            dlsb = singles.tile([P, NBLK * CPB], F32, name="dlsb")
            nc.sync.dma_start(out=dlsb[:], in_=dlt[:])

            # resident self-term tiles: hps = dis * hp = dis^2 * h
            hps1 = singles.tile([P, NBLK, C], F32, name="hps1")
            hps2 = singles.tile([P, NBLK, C], F32, name="hps2")
            if NPC % P != 0:
                # zero once so partial-block tail rows stay zero
                nc.vector.memset(hps1[:], 0.0)
                nc.vector.memset(hps2[:], 0.0)
            def ag_full(agin, hpf):
                nc.gpsimd.collective_compute(
                    "AllGather",
                    mybir.AluOpType.bypass,
                    replica_groups=rg,
                    ins=[agin.opt()],
                    outs=[hpf.opt()],
                )

            # ---------------- stage A: h = mean_t(x) @ W1.T + b1, prescale
            for b in range(NBLK):
                Pb = min(P, NPC - b * P)
                dcol = dissb[:Pb, b : b + 1]
                xt = wp.tile([P, T, C], F32, tag="xt")
                nc.sync.dma_start(out=xt[:Pb], in_=xs[b * P : b * P + Pb])
                s0 = wp.tile([P, C], F32, tag="s0")
                s1 = wp.tile([P, C], F32, tag="s1")
                xm = wp.tile([P, C], BF16, tag="xm")
                nc.vector.tensor_add(out=s0[:Pb], in0=xt[:Pb, 0], in1=xt[:Pb, 1])
                nc.vector.tensor_add(out=s1[:Pb], in0=xt[:Pb, 2], in1=xt[:Pb, 3])
                nc.vector.tensor_add(out=xm[:Pb], in0=s0[:Pb], in1=s1[:Pb])
                hpp = psA.tile([P, C], F32, tag="hpp")
                for k, (c0, cs) in enumerate(CC):
                    ptr = psT.tile([P, P], BF16, tag="ptr")
                    nc.tensor.transpose(
                        out=ptr[:cs, :Pb],
                        in_=xm[:Pb, c0 : c0 + cs],
                        identity=ident[:Pb, :Pb],
                    )
                    xT = wp.tile([P, P], BF16, tag="xT")
                    nc.scalar.copy(out=xT[:cs, :Pb], in_=ptr[:cs, :Pb])
                    nc.tensor.matmul(
                        out=hpp[:Pb],
                        lhsT=xT[:cs, :Pb],
                        rhs=w1sb[:cs, k, :],
                        start=(k == 0),
                        stop=(k == KC - 1),
                    )
                th = wp.tile([P, C], F32, tag="th")
                nc.vector.tensor_add(out=th[:Pb], in0=hpp[:Pb], in1=b1sb[:Pb])
                hp_t = wp.tile([P, C], BF16, tag="hp")
                nc.scalar.activation(out=hp_t[:Pb], in_=th[:Pb], func=ACT.Copy, scale=dcol)
                nc.sync.dma_start(out=agin1[b * P : b * P + Pb], in_=hp_t[:Pb])
                nc.scalar.activation(
                    out=hps1[:Pb, b, :], in_=hp_t[:Pb], func=ACT.Copy, scale=dcol
                )
                if b == NBLK - 1:
                    ag_full(agin1, hp1f)


            # ------------- prop core: gather + indicator matmuls -> psum
            def prop_psum(b, src_full, pool):
                pp = pool.tile([P, C], F32, tag="pp")
                for ch in range(CPB):
                    j = b * CPB + ch
                    msg = mp.tile([P, C], BF16, tag="msg")
                    nc.gpsimd.indirect_dma_start(
                        out=msg[:],
                        out_offset=None,
                        in_=src_full[:],
                        in_offset=IndirectOffsetOnAxis(
                            ap=idxsb[:, j : j + 1], axis=0
                        ),
                    )
                    ind = wp.tile([P, P], BF16, tag="ind")
                    nc.vector.tensor_tensor(
                        out=ind[:],
                        in0=iosb[:],
                        in1=dlsb[:, j : j + 1].to_broadcast([P, P]),
                        op=mybir.AluOpType.is_equal,
                    )
                    nc.tensor.matmul(
                        out=pp[:],
                        lhsT=ind[:],
                        rhs=msg[:],
                        start=(ch == 0),
                        stop=(ch == CPB - 1),
                    )
                return pp

            # ---------------- layer 1 prop + layer 2 linear (fused per block)
            for b in range(NBLK):
                Pb = min(P, NPC - b * P)
                dcol = dissb[:, b : b + 1]
                pp = prop_psum(b, hp1f, psB)
                t1 = wp.tile([P, C], F32, tag="t1")
                nc.vector.scalar_tensor_tensor(
                    out=t1[:],
                    in0=pp[:],
                    scalar=dcol,
                    in1=hps1[:, b, :],
                    op0=mybir.AluOpType.mult,
                    op1=mybir.AluOpType.add,
                )
                h1 = wp.tile([P, C], BF16, tag="h1")
                nc.vector.scalar_tensor_tensor(
                    out=h1[:],
                    in0=t1[:],
                    scalar=0.01,
                    in1=t1[:],
                    op0=mybir.AluOpType.mult,
                    op1=mybir.AluOpType.max,
                )
                h2p = psC.tile([P, C], F32, tag="h2p")
                for k, (c0, cs) in enumerate(CC):
                    ptr2 = psT.tile([P, P], BF16, tag="ptr")
                    nc.tensor.transpose(
                        out=ptr2[:cs, :], in_=h1[:, c0 : c0 + cs], identity=ident[:]
                    )
                    hT = wp.tile([P, P], BF16, tag="hT")
                    nc.scalar.copy(out=hT[:cs, :], in_=ptr2[:cs, :])
                    nc.tensor.matmul(
                        out=h2p[:],
                        lhsT=hT[:cs, :],
                        rhs=w2sb[:cs, k, :],
                        start=(k == 0),
                        stop=(k == KC - 1),
                    )
                t2 = wp.tile([P, C], F32, tag="t2")
                nc.vector.tensor_add(out=t2[:], in0=h2p[:], in1=b2sb[:])
                hp2_t = wp.tile([P, C], BF16, tag="hp2")
                nc.scalar.activation(
                    out=hp2_t[:Pb], in_=t2[:Pb], func=ACT.Copy, scale=dissb[:Pb, b : b + 1]
                )
                nc.sync.dma_start(out=agin2[b * P : b * P + Pb], in_=hp2_t[:Pb])
                nc.scalar.activation(
                    out=hps2[:Pb, b, :],
                    in_=hp2_t[:Pb],
                    func=ACT.Copy,
                    scale=dissb[:Pb, b : b + 1],
                )
                if b == NBLK - 1:
                    ag_full(agin2, hp2f)


            # ---------------- layer 2 prop -> output
            for b in range(NBLK):
                Pb = min(P, NPC - b * P)
                dcol = dissb[:, b : b + 1]
                pp = prop_psum(b, hp2f, psB)
                ot = wp.tile([P, C], F32, tag="ot")
                nc.vector.scalar_tensor_tensor(
                    out=ot[:],
                    in0=pp[:],
                    scalar=dcol,
                    in1=hps2[:, b, :],
                    op0=mybir.AluOpType.mult,
                    op1=mybir.AluOpType.add,
                )
                nc.sync.dma_start(out=out_ext[b * P : b * P + Pb], in_=ot[:Pb])

    nc.compile()
    return nc


# ------------------------------------------------------------------ runner

_CACHE = {}


def run(x, edge_index, W1, b1, W2, b2, n_cores=N_CORES, trace=False):
    in_maps, meta = prep_inputs(x, edge_index, W1, b1, W2, b2, n_cores)
    key = (meta["N"], meta["T"], meta["C"], meta["CPB"], n_cores)
    if key not in _CACHE:
        _CACHE[key] = build_nc(meta)
    nc = _CACHE[key]
    res = bass_utils.run_bass_kernel_spmd(
        nc, in_maps, core_ids=list(range(n_cores)), trace=trace
    )
    NPC = meta["NPC"]
    outs = [np.asarray(res.results[c]["out"]) for c in range(n_cores)]
    full = np.concatenate(outs, axis=0).astype(np.float32)
    return full, res


def kernel(x, edge_index, W1, b1, W2, b2):
    x = np.asarray(x)
    edge_index = np.asarray(edge_index)
    full, _ = run(
        np.asarray(x, np.float32),
        edge_index,
        np.asarray(W1, np.float32),
        np.asarray(b1, np.float32),
        np.asarray(W2, np.float32),
        np.asarray(b2, np.float32),
    )
    return full



# revision 10
# speedup vs baseline: 1.1930x; 1.1930x over previous
"""GCN (2-layer, symmetric-norm message passing) on 8 Trainium2 NeuronCores.

Contract: kernel(**inputs) takes the FULL inputs (x [50000,4,300] f32,
edge_index [2,250000] i32, W1/b1/W2/b2) and returns the FULL output
[50000,300] f32.

Strategy: shard destination nodes across the 8 cores (6250 each), replicate
the small weights, partition edges by destination so scatter-adds are
core-local, and AllGather the pre-scaled source features between layers
(chunked into G groups so the collective overlaps compute).  The scatter-add
runs on the PE array as indicator matmuls over 128-edge chunks (edges sorted
by destination on the host); the per-edge source rows are fetched with
gpsimd.dma_gather (int16 indices, so the gathered table is split in two
halves), whose Q7 descriptor generation (~8.6ns/row) is the critical
resource — index counts are exact (no chunk padding) to minimize it.
"""

import numpy as np

import concourse.bacc as bacc
import concourse.bass as bass
import concourse.tile as tile
from concourse import bass_utils, mybir, library_config
from concourse.masks import make_identity

F32 = mybir.dt.float32
BF16 = mybir.dt.bfloat16
I16 = mybir.dt.int16
P = 128

N_CORES = 8
CE = 384  # gathered row width (bf16) -> 768B, multiple of 256B


def _cdiv(a, b):
    return (a + b - 1) // b


def _r16(a):
    return ((a + 15) // 16) * 16


# ---------------------------------------------------------------- host prep


def prep_inputs(x, edge_index, W1, b1, W2, b2, n_cores=N_CORES, n_groups=2):
    import ml_dtypes

    N, T, C = x.shape
    assert N % n_cores == 0
    NPC = N // n_cores
    NBLK = _cdiv(NPC, P)

    # --- group-major table layout -------------------------------------
    # blocks split into n_groups contiguous runs; table rows are laid out
    # group-major then rank-major so each group's AllGather output is one
    # contiguous slice: trow(c, j) = tbase[g] + c*grows[g] + (j - gstart[g])
    bpg = [NBLK // n_groups + (1 if i < NBLK % n_groups else 0) for i in range(n_groups)]
    gblk0 = np.concatenate([[0], np.cumsum(bpg)])  # block start per group
    gstart = np.minimum(gblk0 * P, NPC)  # row start per group (per core)
    grows = [int(gstart[i + 1] - gstart[i]) for i in range(n_groups)]
    tbase = np.concatenate([[0], np.cumsum([n_cores * r for r in grows])])
    assert tbase[-1] == N

    # permutation: global node id -> table row
    perm = np.zeros(N, np.int64)
    for c in range(n_cores):
        for g in range(n_groups):
            j0, j1 = gstart[g], gstart[g] + grows[g]
            perm[c * NPC + j0 : c * NPC + j1] = (
                tbase[g] + c * grows[g] + np.arange(j1 - j0)
            )
    # halves for int16 indices: groups [0, n_groups/2) and the rest
    hg = n_groups // 2
    half_boundary = int(tbase[hg])
    assert half_boundary < 32768 and (N - half_boundary) < 32768

    row = np.asarray(edge_index[0], dtype=np.int64)
    col = np.asarray(edge_index[1], dtype=np.int64)

    deg = (np.bincount(row, minlength=N) + 1).astype(np.float32)
    dis = (deg**-0.5).astype(np.float32)

    core_of = col // NPC
    src_trow = perm[row]

    # --- per-core, per-block, per-half edge lists ---------------------
    # first pass: counts to derive the uniform (max-over-cores) schedule
    percore = []
    cnts = np.zeros((n_cores, NBLK, 2), np.int64)
    for c in range(n_cores):
        m = core_of == c
        r = src_trow[m]
        d = col[m] - c * NPC
        h = (r >= half_boundary).astype(np.int64)
        order = np.lexsort((d, h, d // P))  # by (block, half, dest)
        r, d, h = r[order], d[order], h[order]
        blk = d // P
        for b in range(NBLK):
            mb = blk == b
            cnts[c, b, 0] = int((h[mb] == 0).sum())
            cnts[c, b, 1] = int((h[mb] == 1).sum())
        percore.append((r, d, h, blk))

    nmax = cnts.max(axis=0)  # [NBLK, 2]
    nidx = np.vectorize(_r16)(nmax)  # idx counts (16-aligned, exact-ish)
    cpb = np.vectorize(lambda v: _cdiv(max(v, 1), P))(nidx)  # chunks per bh
    cpbtot = cpb.sum(axis=1)  # chunks per block
    CPBMAX = int(cpbtot.max())
    TOTCH = int(cpbtot.sum())  # total chunks per layer
    # chunk base offset per block
    chbase = np.concatenate([[0], np.cumsum(cpbtot)])
    # idx column layout: per (b, h): ncols = nidx/16, at col offset icb[b,h]
    icols = nidx // 16
    icb = np.zeros((NBLK, 2), np.int64)
    acc = 0
    for b in range(NBLK):
        for h in range(2):
            icb[b, h] = acc
            acc += icols[b, h]
    TIC = int(acc)

    # --- replicated tensors -------------------------------------------
    CC = [(c0, min(P, C - c0)) for c0 in range(0, C, P)]
    KC = len(CC)
    w1c = np.zeros((KC, P, C), ml_dtypes.bfloat16)
    w2c = np.zeros((KC, P, C), ml_dtypes.bfloat16)
    for k, (c0, cs) in enumerate(CC):
        w1c[k, :cs, :] = (W1.T[c0 : c0 + cs, :] / np.float32(T)).astype(np.float32)
        w2c[k, :cs, :] = W2.T[c0 : c0 + cs, :].astype(np.float32)
    b1t = np.broadcast_to(np.asarray(b1, np.float32), (P, C)).copy()
    b2t = np.broadcast_to(np.asarray(b2, np.float32), (P, C)).copy()
    # wide iota for batched is_equal: [P, CPBMAX*P], col ch*P + d = d
    iot = np.broadcast_to(
        np.tile(np.arange(P, dtype=np.float32), CPBMAX), (P, CPBMAX * P)
    ).copy().astype(ml_dtypes.bfloat16)

    # --- per-core tables ----------------------------------------------
    in_maps = []
    for c in range(n_cores):
        r, d, h, blk = percore[c]
        idxt = np.zeros((16, TIC), np.int16)
        dl2 = np.full((P, TOTCH), -1.0, np.float32)
        for b in range(NBLK):
            mb = blk == b
            for hh in range(2):
                sel = mb & (h == hh)
                rs = r[sel] - (half_boundary if hh else 0)
                ds = (d[sel] - b * P).astype(np.float32)
                n = len(rs)
                ni = int(nidx[b, hh])
                rbuf = np.zeros(ni, np.int64)
                rbuf[:n] = rs
                # idx layout: element i -> [i % 16, col0 + i // 16]
                idxt[:, icb[b, hh] : icb[b, hh] + ni // 16] = rbuf.reshape(
                    ni // 16, 16
                ).T.astype(np.int16)
                # dl layout: slot i -> [i % 128, chunk i // 128]
                nslot = int(cpb[b, hh]) * P
                dbuf = np.full(nslot, -1.0, np.float32)
                dbuf[:n] = ds
                ch0 = chbase[b] + (cpb[b, 0] if hh else 0)
                dl2[:, ch0 : ch0 + cpb[b, hh]] = dbuf.reshape(int(cpb[b, hh]), P).T
        idx_rep = np.tile(idxt, (8, 1))  # replicate across the 8 Q7 cores

        dis_c = dis[c * NPC : (c + 1) * NPC]
        flat = np.zeros(NBLK * P, np.float32)
        flat[:NPC] = dis_c
        dist = flat.reshape(NBLK, P).T.copy()

        in_maps.append(
            {
                "xs": np.ascontiguousarray(x[c * NPC : (c + 1) * NPC]).astype(
                    np.float32
                ),
                "w1c": w1c,
                "w2c": w2c,
                "b1t": b1t,
                "b2t": b2t,
                "iot": iot,
                "dist": dist,
                "idxt": idx_rep,
                "dlt": dl2.astype(ml_dtypes.bfloat16),
            }
        )

    meta = dict(
        N=N,
        T=T,
        C=C,
        NPC=NPC,
        NBLK=NBLK,
        CC=CC,
        n_cores=n_cores,
        n_groups=n_groups,
        gstart=[int(v) for v in gstart],
        grows=grows,
        tbase=[int(v) for v in tbase],
        half_boundary=half_boundary,
        nidx=nidx.tolist(),
        cpb=cpb.tolist(),
        cpbtot=cpbtot.tolist(),
        CPBMAX=CPBMAX,
        TOTCH=TOTCH,
        chbase=chbase.tolist(),
        icb=icb.tolist(),
        icols=icols.tolist(),
        TIC=TIC,
    )
    return in_maps, meta


# ------------------------------------------------------------- device build


def build_nc(meta):
    N = meta["N"]
    T = meta["T"]
    C = meta["C"]
    NPC = meta["NPC"]
    NBLK = meta["NBLK"]
    CC = meta["CC"]
    KC = len(CC)
    n_cores = meta["n_cores"]
    G = meta["n_groups"]
    gstart = meta["gstart"]
    grows = meta["grows"]
    tbase = meta["tbase"]
    HB = meta["half_boundary"]
    nidx = meta["nidx"]
    cpb = meta["cpb"]
    chbase = meta["chbase"]
    icb = meta["icb"]
    CPBMAX = meta["CPBMAX"]
    TOTCH = meta["TOTCH"]
    TIC = meta["TIC"]
    rg = [list(range(n_cores))]

    nc = bacc.Bacc("TRN2", target_bir_lowering=False, debug=False, num_devices=n_cores)

    xs = nc.dram_tensor("xs", [NPC, T, C], F32, kind="ExternalInput")
    w1c = nc.dram_tensor("w1c", [KC, P, C], BF16, kind="ExternalInput")
    w2c = nc.dram_tensor("w2c", [KC, P, C], BF16, kind="ExternalInput")
    b1t = nc.dram_tensor("b1t", [P, C], F32, kind="ExternalInput")
    b2t = nc.dram_tensor("b2t", [P, C], F32, kind="ExternalInput")
    iot = nc.dram_tensor("iot", [P, CPBMAX * P], BF16, kind="ExternalInput")
    dist = nc.dram_tensor("dist", [P, NBLK], F32, kind="ExternalInput")
    idxt = nc.dram_tensor("idxt", [P, TIC], I16, kind="ExternalInput")
    dlt = nc.dram_tensor("dlt", [P, TOTCH], BF16, kind="ExternalInput")
    out_ext = nc.dram_tensor("out", [NPC, C], F32, kind="ExternalOutput")

    ACT = mybir.ActivationFunctionType

    with tile.TileContext(nc) as tc:
        with (
            tc.tile_pool(name="dramp", bufs=1, space="DRAM") as dramp,
            tc.tile_pool(name="singles", bufs=1) as singles,
            tc.tile_pool(name="xload", bufs=4) as xp,
            tc.tile_pool(name="work", bufs=3) as wp,
            tc.tile_pool(name="msgs", bufs=6) as mp,
            tc.tile_pool(name="psA", bufs=2, space="PSUM") as psA,
            tc.tile_pool(name="psT", bufs=2, space="PSUM") as psT,
            tc.tile_pool(name="psB", bufs=2, space="PSUM") as psB,
            tc.tile_pool(name="psC", bufs=2, space="PSUM") as psC,
        ):
            agin1 = dramp.tile([NPC, CE], BF16, name="agin1")
            agin2 = dramp.tile([NPC, CE], BF16, name="agin2")
            # one Shared tensor per (layer, half): a Shared DRAM tensor may
            # only be written by a single collective instruction
            hp1h = [
                dramp.tile(
                    [n_cores * grows[g], CE], BF16, addr_space="Shared",
                    name=f"hp1h{g}",
                )
                for g in range(G)
            ]
            hp2h = [
                dramp.tile(
                    [n_cores * grows[g], CE], BF16, addr_space="Shared",
                    name=f"hp2h{g}",
                )
                for g in range(G)
            ]

            ident = singles.tile([P, P], BF16, name="ident")
            make_identity(nc, ident[:])
            w1sb = singles.tile([P, KC, C], BF16, name="w1sb")
            w2sb = singles.tile([P, KC, C], BF16, name="w2sb")
            for k in range(KC):
                nc.sync.dma_start(out=w1sb[:, k, :], in_=w1c[k])
                nc.sync.dma_start(out=w2sb[:, k, :], in_=w2c[k])
            b1sb = singles.tile([P, C], F32, name="b1sb")
            nc.sync.dma_start(out=b1sb[:], in_=b1t[:])
            b2sb = singles.tile([P, C], F32, name="b2sb")
            nc.sync.dma_start(out=b2sb[:], in_=b2t[:])
            iosb = singles.tile([P, CPBMAX, P], BF16, name="iosb")
            nc.sync.dma_start(out=iosb[:, :, :], in_=iot[:])
            dissb = singles.tile([P, NBLK], F32, name="dissb")
            nc.sync.dma_start(out=dissb[:], in_=dist[:])
            idxsb = singles.tile([P, TIC], I16, name="idxsb")
            nc.sync.dma_start(out=idxsb[:], in_=idxt[:])
            dlsb = singles.tile([P, TOTCH, 1], BF16, name="dlsb")
            nc.sync.dma_start(out=dlsb[:, :, 0], in_=dlt[:])

            # resident self-term tiles (dis^2 * h), bf16
            hps1 = singles.tile([P, NBLK, C], BF16, name="hps1")
            hps2 = singles.tile([P, NBLK, C], BF16, name="hps2")
            nc.vector.memset(hps1[:], 0.0)
            nc.vector.memset(hps2[:], 0.0)

            nc.gpsimd.load_library(library_config.mlp)

            # ---------------- stage A: h = mean_t(x) @ W1.T + b1, prescale
            for b in range(NBLK):
                Pb = min(P, NPC - b * P)
                dcol = dissb[:Pb, b : b + 1]
                xt = xp.tile([P, T * C], F32, tag="xt")
                nc.sync.dma_start(out=xt[:Pb], in_=xs[b * P : b * P + Pb])
                s01 = wp.tile([P, 2 * C], F32, tag="s01")
                nc.vector.tensor_add(
                    out=s01[:Pb], in0=xt[:Pb, : 2 * C], in1=xt[:Pb, 2 * C :]
                )
                xm = wp.tile([P, C], BF16, tag="xm")
                nc.vector.tensor_add(out=xm[:Pb], in0=s01[:Pb, :C], in1=s01[:Pb, C:])
                ptr = psT.tile([P, KC * P], BF16, tag="ptr")
                for k, (c0, cs) in enumerate(CC):
                    nc.tensor.transpose(
                        out=ptr[:cs, k * Pb : (k + 1) * Pb],
                        in_=xm[:Pb, c0 : c0 + cs],
                        identity=ident[:Pb, :Pb],
                    )
                xT = wp.tile([P, KC * P], BF16, tag="xT")
                csL = CC[-1][1]
                nc.scalar.copy(
                    out=xT[:, : (KC - 1) * Pb], in_=ptr[:, : (KC - 1) * Pb]
                )
                nc.scalar.copy(
                    out=xT[:csL, (KC - 1) * Pb : KC * Pb],
                    in_=ptr[:csL, (KC - 1) * Pb : KC * Pb],
                )
                hpp = psA.tile([P, C], F32, tag="hpp")
                for k, (c0, cs) in enumerate(CC):
                    nc.tensor.matmul(
                        out=hpp[:Pb],
                        lhsT=xT[:cs, k * Pb : k * Pb + Pb],
                        rhs=w1sb[:cs, k, :],
                        start=(k == 0),
                        stop=(k == KC - 1),
                    )
                th = wp.tile([P, C], F32, tag="th")
                nc.vector.tensor_add(out=th[:Pb], in0=hpp[:Pb], in1=b1sb[:Pb])
                hp_t = wp.tile([P, CE], BF16, tag="hp")
                nc.vector.memset(hp_t[:Pb, C:], 0.0)
                nc.scalar.activation(
                    out=hp_t[:Pb, :C], in_=th[:Pb], func=ACT.Copy, scale=dcol
                )
                nc.sync.dma_start(
                    out=agin1[b * P : b * P + Pb], in_=hp_t[:Pb]
                )
                nc.scalar.activation(
                    out=hps1[:Pb, b, :], in_=hp_t[:Pb, :C], func=ACT.Copy, scale=dcol
                )

            # ---------------- per-half AllGathers
            def ag_groups(agin, hph):
                for g in range(G):
                    rs, re = gstart[g], gstart[g] + grows[g]
                    nc.gpsimd.collective_compute(
                        "AllGather",
                        mybir.AluOpType.bypass,
                        replica_groups=rg,
                        ins=[agin[rs:re].opt()],
                        outs=[hph[g][:].opt()],
                    )

            ag_groups(agin1, hp1h)

            # ------------- prop core: dma_gather (two halves) + ind matmuls
            def prop_block(b, hph, pool):
                Pb = min(P, NPC - b * P)
                cpbA, cpbB = cpb[b][0], cpb[b][1]
                cpbT = cpbA + cpbB
                niA, niB = nidx[b][0], nidx[b][1]
                msg = mp.tile([P, CPBMAX, CE], BF16, tag="msg")
                if niA > 0:
                    nc.gpsimd.dma_gather(
                        msg[:, :cpbA, :],
                        hph[0][:],
                        idxsb[:, icb[b][0] : icb[b][0] + niA // 16],
                        niA,
                        niA,
                        CE,
                    )
                if niB > 0:
                    nc.gpsimd.dma_gather(
                        msg[:, cpbA : cpbA + cpbB, :],
                        hph[1][:],
                        idxsb[:, icb[b][1] : icb[b][1] + niB // 16],
                        niB,
                        niB,
                        CE,
                    )
                ind = wp.tile([P, CPBMAX, P], BF16, tag="ind")
                nc.vector.tensor_tensor(
                    out=ind[:, :cpbT, :],
                    in0=iosb[:, :cpbT, :],
                    in1=dlsb[:, chbase[b] : chbase[b] + cpbT, :].to_broadcast(
                        [P, cpbT, P]
                    ),
                    op=mybir.AluOpType.is_equal,
                )
                pp = pool.tile([P, C], F32, tag="pp")
                pieces = []
                for ch in range(cpbT):
                    nv = niA - ch * P if ch < cpbA else niB - (ch - cpbA) * P
                    nv = min(P, nv)
                    if nv > 0:
                        pieces.append((ch, nv))
                assert pieces, f"block {b} has no edges"
                for i, (ch, nv) in enumerate(pieces):
                    nc.tensor.matmul(
                        out=pp[:],
                        lhsT=ind[:nv, ch, :],
                        rhs=msg[:nv, ch, :C],
                        start=(i == 0),
                        stop=(i == len(pieces) - 1),
                    )
                return pp, Pb

            # ---------------- layer 1 prop + layer 2 linear (fused per block)
            for b in range(NBLK):
                pp, Pb = prop_block(b, hp1h, psB)
                dcol = dissb[:, b : b + 1]
                t1 = wp.tile([P, C], F32, tag="t1")
                nc.vector.scalar_tensor_tensor(
                    out=t1[:],
                    in0=pp[:],
                    scalar=dcol,
                    in1=hps1[:, b, :],
                    op0=mybir.AluOpType.mult,
                    op1=mybir.AluOpType.add,
                )
                h1 = wp.tile([P, C], BF16, tag="h1")
                nc.vector.scalar_tensor_tensor(
                    out=h1[:],
                    in0=t1[:],
                    scalar=0.01,
                    in1=t1[:],
                    op0=mybir.AluOpType.mult,
                    op1=mybir.AluOpType.max,
                )
                ptr2 = psT.tile([P, KC * P], BF16, tag="ptr")
                for k, (c0, cs) in enumerate(CC):
                    nc.tensor.transpose(
                        out=ptr2[:cs, k * P : (k + 1) * P],
                        in_=h1[:, c0 : c0 + cs],
                        identity=ident[:],
                    )
                hT = wp.tile([P, KC * P], BF16, tag="xT")
                csL = CC[-1][1]
                nc.scalar.copy(
                    out=hT[:, : (KC - 1) * P], in_=ptr2[:, : (KC - 1) * P]
                )
                nc.scalar.copy(
                    out=hT[:csL, (KC - 1) * P :], in_=ptr2[:csL, (KC - 1) * P :]
                )
                h2p = psC.tile([P, C], F32, tag="h2p")
                for k, (c0, cs) in enumerate(CC):
                    nc.tensor.matmul(
                        out=h2p[:],
                        lhsT=hT[:cs, k * P : (k + 1) * P],
                        rhs=w2sb[:cs, k, :],
                        start=(k == 0),
                        stop=(k == KC - 1),
                    )
                t2 = wp.tile([P, C], F32, tag="t2")
                nc.vector.tensor_add(out=t2[:], in0=h2p[:], in1=b2sb[:])
                hp2_t = wp.tile([P, CE], BF16, tag="hp2")
                nc.vector.memset(hp2_t[:Pb, C:], 0.0)
                nc.scalar.activation(
                    out=hp2_t[:Pb, :C], in_=t2[:Pb], func=ACT.Copy, scale=dcol[:Pb]
                )
                nc.sync.dma_start(out=agin2[b * P : b * P + Pb], in_=hp2_t[:Pb])
                nc.scalar.activation(
                    out=hps2[:Pb, b, :],
                    in_=hp2_t[:Pb, :C],
                    func=ACT.Copy,
                    scale=dcol[:Pb],
                )

            ag_groups(agin2, hp2h)

            # ---------------- layer 2 prop -> output
            for b in range(NBLK):
                pp, Pb = prop_block(b, hp2h, psB)
                dcol = dissb[:, b : b + 1]
                ot = wp.tile([P, C], F32, tag="ot")
                nc.vector.scalar_tensor_tensor(
                    out=ot[:],
                    in0=pp[:],
                    scalar=dcol,
                    in1=hps2[:, b, :],
                    op0=mybir.AluOpType.mult,
                    op1=mybir.AluOpType.add,
                )
                nc.sync.dma_start(out=out_ext[b * P : b * P + Pb], in_=ot[:Pb])

    nc.compile()
    return nc


# ------------------------------------------------------------------ runner

_CACHE = {}


def run(x, edge_index, W1, b1, W2, b2, n_cores=N_CORES, trace=False):
    in_maps, meta = prep_inputs(x, edge_index, W1, b1, W2, b2, n_cores)
    key = (
        meta["N"],
        meta["T"],
        meta["C"],
        meta["TOTCH"],
        meta["TIC"],
        tuple(meta["cpbtot"]),
        n_cores,
    )
    if key not in _CACHE:
        _CACHE[key] = build_nc(meta)
    nc = _CACHE[key]
    res = bass_utils.run_bass_kernel_spmd(
        nc, in_maps, core_ids=list(range(n_cores)), trace=trace
    )
    outs = [np.asarray(res.results[c]["out"]) for c in range(n_cores)]
    full = np.concatenate(outs, axis=0).astype(np.float32)
    return full, res


def kernel(x, edge_index, W1, b1, W2, b2):
    full, _ = run(
        np.asarray(x, np.float32),
        np.asarray(edge_index),
        np.asarray(W1, np.float32),
        np.asarray(b1, np.float32),
        np.asarray(W2, np.float32),
        np.asarray(b2, np.float32),
    )
    return full
